# revision 3
# baseline (speedup 1.0000x reference)
"""GCN encoder on 8 Trainium2 NeuronCores.

v2 of the validated 8.47ms kernel. Same host-side layout (nodes
block-sharded 12500/core, per-core ELL message passing via dma_gather
with int16 indices over 4 source windows, per-window in-degree sorts,
width-4 combine gathers, folded normalization, shared mu/logstd
aggregation, AllGather between layers). Device-side changes:

- 4 SWDGE queues with gathers round-robined across them. Measured on
  this HW: one queue drains 4096-desc gathers at ~7.4 ns/desc (the
  128-entry descriptor ring throttles generation to the drain rate);
  4 queues scale to ~1.86 ns/desc.
- 8 message buffers so all 4 queues stay fed.
- Pass reduces batched over same-width runs of groups (one DVE
  instruction per run instead of per group).
- Post-combine scale/relu batched per 14-group chunk using a
  host-built dinv_rep operand; bias adds elided when biases are zero.
Stability envelope kept: single_packet=False, <=8192 idx/gather.
"""

import numpy as np

N = 100000
NC = 8
NLOC = N // NC
G = 98
NPAD = 128 * G
NFULL = NC * NPAD
WIN = NFULL // 4
F1, F2, F3, F4 = 128, 64, 32, 16
CHUNK_COLS = 20
NCHUNK = 14                     # combine chunks of groups
CH = [7] * 14
NQ = 4

_PROFILE = False
_last_exec_ns = None
_TMPDIR = None


def _wrap_idxs(idxs):
    n = len(idxs)
    assert n % 16 == 0
    w = idxs.reshape(n // 16, 16).T.astype(np.int16)
    return np.tile(w, (8, 1))


def _prow(rank):
    return (rank % 128) * G + (rank // 128)


def _preprocess(edge_index):
    src = np.asarray(edge_index[0], dtype=np.int64)
    dst = np.asarray(edge_index[1], dtype=np.int64)
    loop = np.arange(N, dtype=np.int64)
    src = np.concatenate([src, loop])
    dst = np.concatenate([dst, loop])

    deg = np.bincount(dst, minlength=N).astype(np.float64)
    dinv = np.where(deg > 0, 1.0 / np.sqrt(deg), 0.0).astype(np.float32)

    cores = []
    for c in range(NC):
        lo = c * NLOC
        m = (dst >= lo) & (dst < lo + NLOC)
        s_c = src[m]
        d_c = dst[m] - lo
        degloc = np.bincount(d_c, minlength=NLOC)
        order = np.argsort(-degloc, kind="stable")
        rank_of = np.empty(NLOC, np.int64)
        rank_of[order] = np.arange(NLOC)
        cores.append(dict(s=s_c, d=d_c, order=order, rank_of=rank_of))

    row_of_node = np.empty(N, np.int64)
    for c in range(NC):
        rk = cores[c]["rank_of"]
        row_of_node[c * NLOC:(c + 1) * NLOC] = c * NPAD + _prow(rk)

    for c in range(NC):
        cc = cores[c]
        trow = row_of_node[cc["s"]]
        cc["win"] = trow // WIN
        cc["lidx"] = trow % WIN
        # per-window sorts
        cc["order_r"] = []
        cc["rank_r_of"] = []
        cc["deg_r"] = []
        for r in range(4):
            dr = np.bincount(cc["d"][cc["win"] == r], minlength=NLOC)
            o = np.argsort(-dr, kind="stable")
            ro = np.empty(NLOC, np.int64)
            ro[o] = np.arange(NLOC)
            cc["order_r"].append(o)
            cc["rank_r_of"].append(ro)
            cc["deg_r"].append(dr)

    # per-window per-group widths (cross-core max); sorted desc so
    # W_r[g] = max over cores of deg_r[order_r[128*g]]
    Wr = np.zeros((4, G), np.int32)
    for r in range(4):
        for c in range(NC):
            cc = cores[c]
            top = cc["deg_r"][r][cc["order_r"][r][::128][:G]]
            Wr[r] = np.maximum(Wr[r], top)

    zero_local = _prow(NLOC)    # pad-rank row, zero in every table window

    # pass gather index streams
    idx_pass = []               # [core][r] -> [128, 8*sum(Wr[r])]
    for c in range(NC):
        cc = cores[c]
        per_r = []
        for r in range(4):
            m = cc["win"] == r
            d_r = cc["d"][m]
            li_r = cc["lidx"][m]
            rk = cc["rank_r_of"][r][d_r]
            ordk = np.lexsort((li_r, rk))
            rk_s, li_s = rk[ordk], li_r[ordk]
            start = np.searchsorted(rk_s, np.arange(NLOC))
            end = np.searchsorted(rk_s, np.arange(NLOC) + 1)
            parts = []
            for g in range(G):
                w = int(Wr[r][g])
                if w == 0:
                    continue
                seg = np.full((128, w), zero_local, np.int64)
                for p in range(128):
                    rr = 128 * g + p
                    if rr < NLOC:
                        a, b = start[rr], end[rr]
                        if b > a:
                            seg[p, :b - a] = li_s[a:b]
                parts.append(_wrap_idxs(seg.T.reshape(-1)))
            per_r.append(np.concatenate(parts, axis=1) if parts
                         else np.zeros((128, 16), np.int16))
        idx_pass.append(per_r)

    # combine index streams: window pair A=(P0,P1), B=(P2,P3); final order =
    # total-degree ranks. slot i of pair X selects partial of pass 2X+i.
    idx_comb = []               # [core][pair] -> [128, 8*2*G]
    pad_prow = _prow(NLOC)
    for c in range(NC):
        cc = cores[c]
        pair_streams = []
        for pair in range(2):
            parts = []
            for g in range(G):
                seg = np.empty((128, 2), np.int64)
                for i in range(2):
                    r = 2 * pair + i
                    rowv = np.full(128, pad_prow, np.int64)
                    rr = 128 * g + np.arange(128)
                    real = rr < NLOC
                    nodes = cc["order"][rr[real]]
                    rowv[real] = _prow(cc["rank_r_of"][r][nodes])
                    seg[:, i] = rowv + i * NPAD
                parts.append(_wrap_idxs(seg.T.reshape(-1)))
            pair_streams.append(np.concatenate(parts, axis=1))
        idx_comb.append(pair_streams)

    return dinv, cores, Wr, idx_pass, idx_comb


def _build_program(Wr, pass_len, zero_b):
    import contextlib
    import concourse.bacc as bacc
    import concourse.mybir as mybir
    import concourse.tile as tile
    from concourse import library_config
    from concourse.masks import make_identity

    dt = mybir.dt
    Alu = mybir.AluOpType
    nc = bacc.Bacc("TRN2", target_bir_lowering=False, debug=False,
                   num_devices=NC, num_swdge_queues=NQ)

    _q = [0]

    def qn():
        q = _q[0]
        _q[0] = (q + 1) % NQ
        return q

    xT = nc.dram_tensor("xT", [128, NPAD], dt.float32, kind="ExternalInput")
    dinv_d = nc.dram_tensor("dinv", [128, G], dt.float32, kind="ExternalInput")
    w1_d = nc.dram_tensor("w1", [F1, F2], dt.float32, kind="ExternalInput")
    w2_d = nc.dram_tensor("w2", [F2, F3], dt.float32, kind="ExternalInput")
    wmu_d = nc.dram_tensor("wmu", [F3, F4], dt.float32, kind="ExternalInput")
    wls_d = nc.dram_tensor("wls", [F3, F4], dt.float32, kind="ExternalInput")
    b1_d = nc.dram_tensor("b1t", [128, F2], dt.float32, kind="ExternalInput")
    b2_d = nc.dram_tensor("b2t", [128, F3], dt.float32, kind="ExternalInput")
    bmu_d = nc.dram_tensor("bmut", [128, F4], dt.float32, kind="ExternalInput")
    bls_d = nc.dram_tensor("blst", [128, F4], dt.float32, kind="ExternalInput")
    idxp_d = [nc.dram_tensor(f"idxp{r}", [128, pass_len[r]], dt.int16,
                             kind="ExternalInput") for r in range(4)]
    idxc_d = [nc.dram_tensor(f"idxc{p}", [128, 16 * G], dt.int16,
                             kind="ExternalInput") for p in range(2)]
    mu_out = nc.dram_tensor("mu", [128, G, F4], dt.float32,
                            kind="ExternalOutput")
    ls_out = nc.dram_tensor("ls", [128, G, F4], dt.float32,
                            kind="ExternalOutput")

    FW = F2

    with tile.TileContext(nc) as tc:
        with contextlib.ExitStack() as ctx:
            dram = ctx.enter_context(
                tc.tile_pool(name="dram", bufs=1, space="DRAM"))
            consts = ctx.enter_context(tc.tile_pool(name="consts", bufs=1))
            psum_mm = ctx.enter_context(
                tc.tile_pool(name="psum_mm", bufs=4, space="PSUM"))
            psum_tr = ctx.enter_context(
                tc.tile_pool(name="psum_tr", bufs=3, space="PSUM"))
            tabp = ctx.enter_context(tc.tile_pool(name="tabp", bufs=1))
            psbp = ctx.enter_context(tc.tile_pool(name="psbp", bufs=2))
            aggp = ctx.enter_context(tc.tile_pool(name="aggp", bufs=1))
            smallp = ctx.enter_context(tc.tile_pool(name="smallp", bufs=6))

            nc.gpsimd.load_library(library_config.mlp)

            def cload(name, dram_t, shape):
                t = consts.tile(shape, dt.float32, name=name)
                nc.sync.dma_start(t[:], dram_t[:])
                return t

            dinv_sb = cload("dinv_sb", dinv_d, [128, G])
            w1_sb = cload("w1_sb", w1_d, [F1, F2])
            w2_sb = cload("w2_sb", w2_d, [F2, F3])
            wmu_sb = cload("wmu_sb", wmu_d, [F3, F4])
            wls_sb = cload("wls_sb", wls_d, [F3, F4])
            b1_sb = cload("b1_sb", b1_d, [128, F2])
            b2_sb = cload("b2_sb", b2_d, [128, F3])
            bmu_sb = cload("bmu_sb", bmu_d, [128, F4])
            bls_sb = cload("bls_sb", bls_d, [128, F4])
            ident = consts.tile([128, 128], dt.float32, name="ident")
            make_identity(nc, ident[:])

            def store_table(tab_sb, name):
                loc = dram.tile([NPAD, FW], dt.float32, name=name)
                nc.sync.dma_start(
                    loc[:].rearrange("(p g) f -> p g f", p=128), tab_sb[:])
                full = dram.tile([NFULL, FW], dt.float32,
                                 addr_space="Shared", name=name + "_full")
                nc.gpsimd.collective_compute(
                    "AllGather", Alu.bypass,
                    replica_groups=[list(range(NC))],
                    ins=[loc.opt()], outs=[full.opt()],
                )
                return full

            # ---------- Layer 1 matmul ----------
            with tc.tile_pool(name="xTp", bufs=1) as xp:
                xT_sb = xp.tile([128, NPAD], dt.float32, name="xT_sb")
                nc.sync.dma_start(xT_sb[:], xT[:])
                tab_sb = tabp.tile([128, G, FW], dt.float32, tag="tab",
                                   name="tab1_sb")
                for g in range(G):
                    ps = psum_mm.tile([128, FW], dt.float32, space="PSUM",
                                      tag="mm", name=f"mm1_{g}")
                    nc.tensor.matmul(out=ps[:],
                                     lhsT=xT_sb[:, 128 * g:128 * (g + 1)],
                                     rhs=w1_sb[:], start=True, stop=True)
                    nc.vector.tensor_scalar_mul(
                        tab_sb[:, g, :], ps[:], dinv_sb[:, g:g + 1])
                tab1_full = store_table(tab_sb, "tab1")

            idxp = ctx.enter_context(tc.tile_pool(name="idxp", bufs=1))
            msgp = ctx.enter_context(tc.tile_pool(name="msgp", bufs=8))
            combp = ctx.enter_context(tc.tile_pool(name="combp", bufs=4))
            combrp = ctx.enter_context(tc.tile_pool(name="combrp", bufs=2))
            idx_sb = {}
            for r in range(4):
                t = idxp.tile([128, pass_len[r]], dt.int16, tag=f"idx{r}",
                              name=f"idxt{r}")
                nc.sync.dma_start(t[:], idxp_d[r][:])
                idx_sb[r] = t
            idxc_sb = {}
            for p in range(2):
                t = idxp.tile([128, 16 * G], dt.int16, tag=f"idxc{p}",
                              name=f"idxct{p}")
                nc.sync.dma_start(t[:], idxc_d[p][:])
                idxc_sb[p] = t

            def aggregate(tab_full, out_chunk_cb, phase):
                # 4 window passes into partial tables
                pairs = []
                for pair in range(2):
                    pab = dram.tile([2 * NPAD, FW], dt.float32,
                                    name=f"pab_{phase}_{pair}")
                    pairs.append(pab)
                for r in range(4):
                    P_sb = psbp.tile([128, G, FW], dt.float32, tag="psb",
                                      name=f"psb_{phase}_{r}")
                    for g0 in range(G):
                        if int(Wr[r][g0]) == 0:
                            nc.vector.memset(P_sb[:, g0, :], 0.0)
                    # pack whole groups into <=CHUNK_COLS-column chunks
                    chunks = []
                    cur, cols = [], 0
                    for g in range(G):
                        w = int(Wr[r][g])
                        if w == 0:
                            continue
                        assert w <= CHUNK_COLS, (r, g, w)
                        if cols + w > CHUNK_COLS:
                            chunks.append((cur, cols))
                            cur, cols = [], 0
                        cur.append((g, w, cols))
                        cols += w
                    if cur:
                        chunks.append((cur, cols))
                    off = 0
                    for ci, (members, cols) in enumerate(chunks):
                        mt = msgp.tile([128, cols, FW], dt.float32,
                                       tag="msg",
                                       name=f"m_{phase}_{r}_{ci}")
                        nc.gpsimd.dma_gather(
                            mt[:], tab_full[r * WIN:(r + 1) * WIN, :],
                            idx_sb[r][:, off:off + 8 * cols],
                            128 * cols, 128 * cols, FW,
                            single_packet=False, queue_num=qn(),
                        )
                        off += 8 * cols
                        # batch same-width runs of consecutive groups
                        runs = []
                        for (g, w, co) in members:
                            if (runs and runs[-1][1] == w
                                    and runs[-1][0] + len(runs[-1][2]) == g):
                                runs[-1][2].append(g)
                            else:
                                runs.append([g, w, [g], co])
                        for (g0r, w, gs, co) in runs:
                            k = len(gs)
                            nc.vector.tensor_reduce(
                                P_sb[:, g0r:g0r + k, :],
                                mt[:, co:co + k * w, :]
                                .rearrange("p (gg w) f -> p gg f w", w=w),
                                axis=mybir.AxisListType.X, op=Alu.add)
                    nc.sync.dma_start(
                        pairs[r // 2][(r % 2) * NPAD:(r % 2 + 1) * NPAD, :]
                        .rearrange("(p g) f -> p g f", p=128),
                        P_sb[:])
                # combine: width-4 gather over the two pair tables
                gl0 = 0
                for ci in range(NCHUNK):
                    ng = CH[ci]
                    outs = []
                    for pair in range(2):
                        mt = combp.tile([128, 2 * ng, FW], dt.float32,
                                        tag="cmb",
                                        name=f"cm_{phase}_{ci}_{pair}")
                        nc.gpsimd.dma_gather(
                            mt[:], pairs[pair][:, :],
                            idxc_sb[pair][:, 16 * gl0:16 * (gl0 + ng)],
                            128 * 2 * ng, 128 * 2 * ng, FW,
                            single_packet=False, queue_num=qn(),
                        )
                        red = combrp.tile([128, ng, FW], dt.float32,
                                          tag="crd",
                                         name=f"cr_{phase}_{ci}_{pair}")
                        nc.vector.tensor_reduce(
                            red[:],
                            mt[:].rearrange("p (g two) f -> p g f two",
                                            two=2),
                            axis=mybir.AxisListType.X, op=Alu.add)
                        outs.append(red)
                    comb = combrp.tile([128, ng, FW], dt.float32,
                                       tag="cfin", name=f"cf_{phase}_{ci}")
                    nc.vector.tensor_tensor(comb[:], outs[0][:], outs[1][:],
                                            op=Alu.add)
                    out_chunk_cb(ci, gl0, ng, comb)
                    gl0 += ng

            # ---------- Layer 1 aggregate -> x1 ----------
            x1_cs = [aggp.tile([128, CH[ci], F2], dt.float32,
                               tag=f"x1_{ci}", name=f"x1_sb{ci}")
                     for ci in range(NCHUNK)]

            def l1_chunk(ci, gl0, ng, comb):
                nc.vector.tensor_tensor(
                    comb[:], comb[:],
                    dinv_sb[:, gl0:gl0 + ng].to_broadcast([128, ng, F2]),
                    op=Alu.mult)
                if not zero_b:
                    for gi in range(ng):
                        nc.vector.tensor_tensor(comb[:, gi, :],
                                                comb[:, gi, :],
                                                b1_sb[:], op=Alu.add)
                nc.vector.tensor_scalar(x1_cs[ci][:], comb[:],
                                        0.0, None, Alu.max)

            aggregate(tab1_full, l1_chunk, "l1")

            # ---------- Layer 2 ----------
            tab_sb2 = tabp.tile([128, G, FW], dt.float32, tag="tab",
                                name="tab2_sb")
            nc.vector.memset(tab_sb2[:], 0.0)
            for g in range(G):
                pt = psum_tr.tile([F2, 128], dt.float32, space="PSUM",
                                  tag="tr", name=f"tr2_{g}")
                nc.tensor.transpose(pt[:], x1_cs[g // 7][:, g % 7, :],
                                    ident[:])
                x1t = smallp.tile([F2, 128], dt.float32, tag="x1t",
                                  name=f"x1t_{g}")
                nc.vector.tensor_copy(x1t[:], pt[:])
                ps = psum_mm.tile([128, FW], dt.float32, space="PSUM",
                                  tag="mm", name=f"mm2_{g}")
                nc.tensor.matmul(out=ps[:, 0:F3], lhsT=x1t[:], rhs=w2_sb[:],
                                 start=True, stop=True)
                nc.vector.tensor_scalar_mul(
                    tab_sb2[:, g, 0:F3], ps[:, 0:F3], dinv_sb[:, g:g + 1])
            tab2_full = store_table(tab_sb2, "tab2")

            x2_cs = [aggp.tile([128, CH[ci], F3], dt.float32,
                               tag=f"x1_{ci}", name=f"x2_sb{ci}")
                     for ci in range(NCHUNK)]

            def l2_chunk(ci, gl0, ng, comb):
                nc.vector.tensor_tensor(
                    comb[:, :, 0:F3], comb[:, :, 0:F3],
                    dinv_sb[:, gl0:gl0 + ng].to_broadcast([128, ng, F3]),
                    op=Alu.mult)
                if not zero_b:
                    for gi in range(ng):
                        nc.vector.tensor_tensor(comb[:, gi, 0:F3],
                                                comb[:, gi, 0:F3],
                                                b2_sb[:], op=Alu.add)
                nc.vector.tensor_scalar(x2_cs[ci][:], comb[:, :, 0:F3],
                                        0.0, None, Alu.max)

            aggregate(tab2_full, l2_chunk, "l2")

            # ---------- Layer 3 ----------
            tab_sb3 = tabp.tile([128, G, FW], dt.float32, tag="tab",
                                name="tab3_sb")
            nc.vector.memset(tab_sb3[:], 0.0)
            for g in range(G):
                nc.vector.tensor_scalar_mul(
                    tab_sb3[:, g, 0:F3], x2_cs[g // 7][:, g % 7, :],
                    dinv_sb[:, g:g + 1])
            tab3_full = store_table(tab_sb3, "tab3")

            def l3_chunk(ci, gl0, ng, comb):
                mu_c = smallp.tile([128, ng, F4], dt.float32, tag="muc",
                                   name=f"mu_c{ci}")
                ls_c = smallp.tile([128, ng, F4], dt.float32, tag="lsc",
                                   name=f"ls_c{ci}")
                nc.vector.tensor_tensor(
                    comb[:, :, 0:F3], comb[:, :, 0:F3],
                    dinv_sb[:, gl0:gl0 + ng].to_broadcast([128, ng, F3]),
                    op=Alu.mult)
                for gi in range(ng):
                    g = gl0 + gi
                    pt = psum_tr.tile([F3, 128], dt.float32, space="PSUM",
                                      tag="tr", name=f"tr3_{g}")
                    nc.tensor.transpose(pt[:], comb[:, gi, 0:F3], ident[:])
                    zt = smallp.tile([F3, 128], dt.float32, tag="x1t",
                                     name=f"zt_{g}")
                    nc.vector.tensor_copy(zt[:], pt[:])
                    pmu = psum_mm.tile([128, FW], dt.float32, space="PSUM",
                                       tag="mm", name=f"pmu_{g}")
                    nc.tensor.matmul(out=pmu[:, 0:F4], lhsT=zt[:],
                                     rhs=wmu_sb[:], start=True, stop=True)
                    pls = psum_mm.tile([128, FW], dt.float32, space="PSUM",
                                       tag="mm", name=f"pls_{g}")
                    nc.tensor.matmul(out=pls[:, 0:F4], lhsT=zt[:],
                                     rhs=wls_sb[:], start=True, stop=True)
                    if zero_b:
                        nc.vector.tensor_copy(mu_c[:, gi, :], pmu[:, 0:F4])
                        nc.vector.tensor_copy(ls_c[:, gi, :], pls[:, 0:F4])
                    else:
                        nc.vector.tensor_tensor(mu_c[:, gi, :], pmu[:, 0:F4],
                                                bmu_sb[:], op=Alu.add)
                        nc.vector.tensor_tensor(ls_c[:, gi, :], pls[:, 0:F4],
                                                bls_sb[:], op=Alu.add)
                nc.sync.dma_start(mu_out[:, gl0:gl0 + ng, :], mu_c[:])
                nc.sync.dma_start(ls_out[:, gl0:gl0 + ng, :], ls_c[:])

            aggregate(tab3_full, l3_chunk, "l3")

    nc.compile()
    return nc


def kernel(x, edge_index, W1, b1, W2, b2, Wmu, bmu, Wls, bls):
    global _last_exec_ns
    x = np.asarray(x, np.float32)
    dinv, cores, Wr, idx_pass, idx_comb = _preprocess(edge_index)
    pass_len = [idx_pass[0][r].shape[1] for r in range(4)]

    zero_b = not (np.any(np.asarray(b1)) or np.any(np.asarray(b2))
                  or np.any(np.asarray(bmu)) or np.any(np.asarray(bls)))
    nc = _build_program(Wr, pass_len, zero_b)

    def btile(b):
        return np.tile(np.asarray(b, np.float32)[None, :], (128, 1))

    in_maps = []
    for c in range(NC):
        cc = cores[c]
        xT = np.zeros((128, NPAD), np.float32)
        xT[:, cc["rank_of"]] = x[c * NLOC:(c + 1) * NLOC].T

        dv = np.zeros((128, G), np.float32)
        rr = np.arange(128)[:, None] + 128 * np.arange(G)[None, :]
        mreal = rr < NLOC
        dv[mreal] = dinv[c * NLOC + cc["order"][rr[mreal]]]

        im = dict(xT=xT, dinv=dv,
                  w1=np.asarray(W1, np.float32),
                  w2=np.asarray(W2, np.float32),
                  wmu=np.asarray(Wmu, np.float32),
                  wls=np.asarray(Wls, np.float32),
                  b1t=btile(b1), b2t=btile(b2), bmut=btile(bmu),
                  blst=btile(bls))
        for r in range(4):
            im[f"idxp{r}"] = idx_pass[c][r]
        for p in range(2):
            im[f"idxc{p}"] = idx_comb[c][p]
        in_maps.append(im)

    from concourse.bass_utils import run_bass_kernel_spmd
    res = run_bass_kernel_spmd(nc, in_maps, core_ids=list(range(NC)),
                               trace=_PROFILE, tmpdir=_TMPDIR)
    _last_exec_ns = res.exec_time_ns

    mu = np.empty((N, F4), np.float32)
    ls = np.empty((N, F4), np.float32)
    rr = np.arange(128)[:, None] + 128 * np.arange(G)[None, :]
    mreal = rr < NLOC
    for c in range(NC):
        mo = np.asarray(res.results[c]["mu"]).reshape(128, G, F4)
        lo = np.asarray(res.results[c]["ls"]).reshape(128, G, F4)
        nodes = c * NLOC + cores[c]["order"][rr[mreal]]
        mu[nodes] = mo[mreal]
        ls[nodes] = lo[mreal]
    return mu, ls

